# revision 1
# baseline (speedup 1.0000x reference)
"""Trainium2 Bass kernel for nn_D_loss_67551245631962.

Computes: 0.8 * sum(WMA5(target_angle - pred_angle)^2) + 0.2 * sum((target_class - pred_class)^2)
where WMA5 is a 5-tap [0.05, 0.1, 0.7, 0.1, 0.05] correlation with 2-zero padding per side.

Strategy (pure data parallelism over batch dim B=2048 across 8 cores, 256 rows/core):
  - Both angle inputs stream in via SWDGE (gpsimd) cast DMAs (fp32 DRAM -> fp16
    SBUF), fully independent so the DMA engines saturate from t=0.
  - DVE: diff = ta - pa as one fp16 tensor_tensor sub (2x mode), then the
    symmetric 5-tap conv s = p14 + 2*(d1+d3) + (d0+d4) as 4 TT adds (2x) +
    1 tensor_scalar mul (4x); wma = 0.05*s with 0.05 folded into host scale.
    HW-verified: fp16 TT=2x even at odd-element offsets; STT is 1x (avoid).
  - ACT: p14 = Copy(14*d2) scaling pass + fused Square + accum_out reduction.
  - Host sums 8 cores' [128, NACC] partials in float64, scales 0.8*0.05^2 / 0.2.
  - Emission order software-pipelines phases (loads | subs | p14s | conv+square)
    so each engine's in-order stream never round-trips another engine per tile.

Per-core engine budget (target memory-bound): DMA ~47-55us (16.8 MB HBM read),
DVE 2.75f ~50us, ACT 2f ~33us.
"""

import os
import sys

for _p in ("/opt/trn_rl_repo",):
    if os.path.isdir(_p) and _p not in sys.path:
        sys.path.insert(0, _p)

from contextlib import ExitStack

import numpy as np

import concourse.bass as bass
import concourse.tile as tile
from concourse import bacc, mybir
from concourse.bass_utils import run_bass_kernel_spmd

N_CORES = 8
B, T = 2048, 8192
RPC = B // N_CORES  # rows per core = 256
G = RPC // 128      # 128-partition row groups per core = 2
F = 2048            # free-dim tile size (conv output cols per tile)
NT = T // F         # column tiles per group
NACC = G * NT + G   # accumulator columns: G*NT angle + G class

W = (0.05, 0.1, 0.7, 0.1, 0.05)

DT16 = mybir.dt.float16  # conv compute storage dtype (2-byte => DVE 2x mode)


def build_nc():
    nc = bacc.Bacc("TRN2")
    dt = mybir.dt
    ta = nc.dram_tensor("target_angle", [RPC, T], dt.float32, kind="ExternalInput")
    pa = nc.dram_tensor("pred_angle", [RPC, T], dt.float32, kind="ExternalInput")
    tcl = nc.dram_tensor("target_class", [RPC, 3], dt.float32, kind="ExternalInput")
    pcl = nc.dram_tensor("pred_class", [RPC, 3], dt.float32, kind="ExternalInput")
    out = nc.dram_tensor("out", [128, NACC], dt.float32, kind="ExternalOutput")

    AF = mybir.ActivationFunctionType
    OP = mybir.AluOpType

    with tile.TileContext(nc) as tc, ExitStack() as ctx:
        tpool = ctx.enter_context(tc.tile_pool(name="dta", bufs=4))
        qpool = ctx.enter_context(tc.tile_pool(name="dpa", bufs=4))
        dpool = ctx.enter_context(tc.tile_pool(name="dbf", bufs=G * NT))
        fpool = ctx.enter_context(tc.tile_pool(name="p14", bufs=G * NT))
        spool = ctx.enter_context(tc.tile_pool(name="s", bufs=6))
        jpool = ctx.enter_context(tc.tile_pool(name="junk", bufs=2))
        apool = ctx.enter_context(tc.tile_pool(name="acc", bufs=1))
        cpool = ctx.enter_context(tc.tile_pool(name="cls", bufs=2))

        accums = apool.tile([128, NACC], dt.float32)

        # tile geometry: tile (g,t) covers diff cols [t*F-2, t*F+F+2) w/ halo
        def geom(t):
            lo, hi = t * F - 2, t * F + F + 2
            dst_lo, dst_hi = 0, F + 4
            if lo < 0:
                dst_lo, lo = 2, 0
            if hi > T:
                dst_hi, hi = F + 2, T
            return lo, hi, dst_lo, dst_hi

        # Phase A: independent SWDGE cast loads of both inputs (fp32->fp16)
        pairs = []
        for g in range(G):
            r0, r1_ = g * 128, (g + 1) * 128
            for t in range(NT):
                lo, hi, dst_lo, dst_hi = geom(t)
                dta = tpool.tile([128, F + 4], DT16, tag="dta")
                dpa = qpool.tile([128, F + 4], DT16, tag="dpa")
                if dst_lo:
                    nc.vector.memset(dta[:, 0:dst_lo], 0.0)
                    nc.vector.memset(dpa[:, 0:dst_lo], 0.0)
                if dst_hi < F + 4:
                    nc.vector.memset(dta[:, dst_hi : F + 4], 0.0)
                    nc.vector.memset(dpa[:, dst_hi : F + 4], 0.0)
                nc.gpsimd.dma_start(dta[:, dst_lo:dst_hi], ta[r0:r1_, lo:hi])
                nc.gpsimd.dma_start(dpa[:, dst_lo:dst_hi], pa[r0:r1_, lo:hi])
                pairs.append((dta, dpa))

        # Phase B1: fp16 diff on DVE (TT sub, 2x)
        dbfs = []
        for dta, dpa in pairs:
            dbf = dpool.tile([128, F + 4], DT16, tag="dbf")
            nc.vector.tensor_sub(dbf[:], dta[:], dpa[:])
            dbfs.append(dbf)

        # Phase B2: p14 = 14*d2 scaling pass on ACT (frees DVE cycles)
        p14s = []
        for dbf in dbfs:
            p14 = fpool.tile([128, F], DT16, tag="p14")
            nc.scalar.activation(p14[:], dbf[:, 2 : F + 2], AF.Copy, scale=14.0)
            p14s.append(p14)

        # Phase B3: conv tail on DVE + fused Square/accum on ACT
        # s = p14 + 2*(d1+d3) + (d0+d4); wma = 0.05*s (host-folded)
        for i, (dbf, p14) in enumerate(zip(dbfs, p14s)):
            u = spool.tile([128, F], DT16, tag="s")
            nc.vector.tensor_add(u[:], dbf[:, 1 : F + 1], dbf[:, 3 : F + 3])
            y = spool.tile([128, F], DT16, tag="s")
            nc.vector.tensor_scalar_mul(y[:], u[:], 2.0)
            v = spool.tile([128, F], DT16, tag="s")
            nc.vector.tensor_add(v[:], dbf[:, 0:F], dbf[:, 4 : F + 4])
            x = spool.tile([128, F], DT16, tag="s")
            nc.vector.tensor_add(x[:], p14[:], y[:])
            s4 = spool.tile([128, F], DT16, tag="s")
            nc.vector.tensor_add(s4[:], x[:], v[:])

            junk = jpool.tile([128, F], DT16, tag="junk")
            nc.scalar.activation(
                junk[:], s4[:], AF.Square, accum_out=accums[:, i : i + 1]
            )

        # class SSE per row group (tiny)
        for g in range(G):
            r0, r1_ = g * 128, (g + 1) * 128
            ct = cpool.tile([128, 3], dt.float32, tag="cls")
            cp = cpool.tile([128, 3], dt.float32, tag="clsp")
            nc.sync.dma_start(ct[:], tcl[r0:r1_, :])
            nc.sync.dma_start(cp[:], pcl[r0:r1_, :])
            cd = cpool.tile([128, 3], dt.float32, tag="clsd")
            nc.vector.tensor_sub(cd[:], ct[:], cp[:])
            cj = cpool.tile([128, 3], dt.float32, tag="clsj")
            col = G * NT + g
            nc.scalar.activation(
                cj[:], cd[:], AF.Square, accum_out=accums[:, col : col + 1]
            )

        nc.sync.dma_start(out[:], accums[:])

    nc.finalize()
    return nc


_NC = None
last_result = None  # BassKernelResults of the most recent run (for test harness)


def kernel(target_angle, pred_angle, target_class, pred_class):
    global _NC, last_result
    if _NC is None:
        _NC = build_nc()

    in_maps = []
    for c in range(N_CORES):
        r = slice(c * RPC, (c + 1) * RPC)
        in_maps.append(
            {
                "target_angle": np.ascontiguousarray(target_angle[r], dtype=np.float32),
                "pred_angle": np.ascontiguousarray(pred_angle[r], dtype=np.float32),
                "target_class": np.ascontiguousarray(target_class[r], dtype=np.float32),
                "pred_class": np.ascontiguousarray(pred_class[r], dtype=np.float32),
            }
        )

    last_result = run_bass_kernel_spmd(
        _NC,
        in_maps,
        core_ids=list(range(N_CORES)),
        trace=bool(os.environ.get("BASS_TRACE")),
    )

    angle = 0.0
    cls = 0.0
    for r in last_result.results:
        o = np.asarray(r["out"], dtype=np.float64)
        angle += o[:, 0 : G * NT].sum()
        cls += o[:, G * NT : NACC].sum()

    val = 0.8 * (W[4] * W[4]) * angle + 0.2 * cls
    return np.array(val, dtype=np.float32)



# revision 2
# speedup vs baseline: 1.0061x; 1.0061x over previous
"""Trainium2 Bass kernel for nn_D_loss_67551245631962.

Computes: 0.8 * sum(WMA5(target_angle - pred_angle)^2) + 0.2 * sum((target_class - pred_class)^2)
where WMA5 is a 5-tap [0.05, 0.1, 0.7, 0.1, 0.05] correlation with 2-zero padding per side.

Strategy (pure data parallelism over batch dim B=2048 across 8 cores, 256 rows/core):
  Filter factorization: [1,2,14,2,1] = [1,1](*)[1,1](*)[1,0,1] + 12*delta_2, so per
  tile (halo'd by 2 cols each side):
    d = ta - pa          (GpSimd TT for the first tiles, DVE TT for the rest)
    u = d0 + d1          (DVE TT, fp16 2x)
    v = u0 + u1          (DVE TT)      -> v = [1,2,1] (*) d
    w = v0 + v2          (DVE TT)      -> w = [1,2,2,2,1] (*) d
    p12 = 12 * d2        (ACT Copy scale)
    s = w + p12          (DVE TT)      -> s = [1,2,14,2,1] (*) d = wma / 0.05
    sum(s^2)             (ACT Square + accum_out, fp32 accumulator column)
  Loads are SWDGE cast DMAs (fp32 DRAM -> fp16 SBUF) emitted all up-front so the
  16 DMA engines stream continuously; 0.05^2*0.8 / 0.2 applied in the host sum.
"""

import os
import sys

for _p in ("/opt/trn_rl_repo",):
    if os.path.isdir(_p) and _p not in sys.path:
        sys.path.insert(0, _p)

from contextlib import ExitStack

import numpy as np

import concourse.bass as bass
import concourse.tile as tile
from concourse import bacc, mybir
from concourse.bass_utils import run_bass_kernel_spmd

N_CORES = 8
B, T = 2048, 8192
RPC = B // N_CORES  # rows per core = 256
G = RPC // 128      # 128-partition row groups per core = 2
F = 4096            # free-dim tile size (conv output cols per tile)
NT = T // F         # column tiles per group = 2
NACC = G * NT + G   # accumulator columns: G*NT angle + G class

DT16 = mybir.dt.float16

# which tiles get their subtraction on GpSimd (load order index)
GPSIMD_SUB_TILES = int(os.environ.get("K_GPS_SUBS", "2"))


def build_nc():
    nc = bacc.Bacc("TRN2")
    dt = mybir.dt
    ta = nc.dram_tensor("target_angle", [RPC, T], dt.float32, kind="ExternalInput")
    pa = nc.dram_tensor("pred_angle", [RPC, T], dt.float32, kind="ExternalInput")
    tcl = nc.dram_tensor("target_class", [RPC, 3], dt.float32, kind="ExternalInput")
    pcl = nc.dram_tensor("pred_class", [RPC, 3], dt.float32, kind="ExternalInput")
    out = nc.dram_tensor("out", [128, NACC], dt.float32, kind="ExternalOutput")

    AF = mybir.ActivationFunctionType
    OP = mybir.AluOpType

    with tile.TileContext(nc) as tc, ExitStack() as ctx:
        apool = ctx.enter_context(tc.tile_pool(name="lda", bufs=4))
        bpool = ctx.enter_context(tc.tile_pool(name="ldb", bufs=4))
        dpool = ctx.enter_context(tc.tile_pool(name="d", bufs=3))
        upool = ctx.enter_context(tc.tile_pool(name="u", bufs=2))
        vpool = ctx.enter_context(tc.tile_pool(name="v", bufs=2))
        wpool = ctx.enter_context(tc.tile_pool(name="w", bufs=2))
        ppool = ctx.enter_context(tc.tile_pool(name="p12", bufs=2))
        spool = ctx.enter_context(tc.tile_pool(name="s", bufs=2))
        jpool = ctx.enter_context(tc.tile_pool(name="junk", bufs=2))
        kpool = ctx.enter_context(tc.tile_pool(name="acc", bufs=1))
        cpool = ctx.enter_context(tc.tile_pool(name="cls", bufs=2))

        accums = kpool.tile([128, NACC], dt.float32)

        # load order: alternate groups so both row-groups stream early
        order = []
        for t in range(NT):
            for g in range(G):
                order.append((g, t))

        def geom(t):
            # tile t covers conv outputs [t*F, t*F+F); needs d cols [t*F-2, t*F+F+2)
            lo, hi = t * F - 2, t * F + F + 2
            dst_lo, dst_hi = 0, F + 4
            if lo < 0:
                dst_lo, lo = 2, 0
            if hi > T:
                dst_hi, hi = F + 2, T
            return lo, hi, dst_lo, dst_hi

        # Phase A: all cast loads up-front (gpsimd SWDGE queue)
        tiles = {}
        for (g, t) in order:
            r0, r1 = g * 128, (g + 1) * 128
            lo, hi, dst_lo, dst_hi = geom(t)
            A = apool.tile([128, F + 4], DT16, tag="A")
            Bt = bpool.tile([128, F + 4], DT16, tag="B")
            if dst_lo:
                nc.vector.memset(A[:, 0:dst_lo], 0.0)
                nc.vector.memset(Bt[:, 0:dst_lo], 0.0)
            if dst_hi < F + 4:
                nc.vector.memset(A[:, dst_hi : F + 4], 0.0)
                nc.vector.memset(Bt[:, dst_hi : F + 4], 0.0)
            nc.gpsimd.dma_start(A[:, dst_lo:dst_hi], ta[r0:r1, lo:hi])
            nc.gpsimd.dma_start(Bt[:, dst_lo:dst_hi], pa[r0:r1, lo:hi])
            tiles[(g, t)] = (A, Bt)

        # class loads early (HWDGE sync queue, fp32)
        cls_tiles = []
        for g in range(G):
            r0, r1 = g * 128, (g + 1) * 128
            ct = cpool.tile([128, 3], dt.float32, tag="ct")
            cp = cpool.tile([128, 3], dt.float32, tag="cp")
            nc.sync.dma_start(ct[:], tcl[r0:r1, :])
            nc.sync.dma_start(cp[:], pcl[r0:r1, :])
            cls_tiles.append((ct, cp))

        # Phase B: subs — first GPSIMD_SUB_TILES tiles on gpsimd, rest on DVE
        dtiles = {}
        for i, (g, t) in enumerate(order):
            A, Bt = tiles[(g, t)]
            d = dpool.tile([128, F + 4], DT16, tag="d")
            if i < GPSIMD_SUB_TILES:
                nc.gpsimd.tensor_tensor(d[:], A[:], Bt[:], OP.subtract)
            else:
                nc.vector.tensor_sub(d[:], A[:], Bt[:])
            dtiles[(g, t)] = d

        # Phase C: cascade + p12 + square per tile
        for i, (g, t) in enumerate(order):
            d = dtiles[(g, t)]
            col = g * NT + t
            p12 = ppool.tile([128, F], DT16, tag="p12")
            nc.scalar.activation(p12[:], d[:, 2 : F + 2], AF.Copy, scale=12.0)
            u = upool.tile([128, F + 3], DT16, tag="u")
            nc.vector.tensor_add(u[:], d[:, 0 : F + 3], d[:, 1 : F + 4])
            v = vpool.tile([128, F + 2], DT16, tag="v")
            nc.vector.tensor_add(v[:], u[:, 0 : F + 2], u[:, 1 : F + 3])
            w = wpool.tile([128, F], DT16, tag="w")
            nc.vector.tensor_add(w[:], v[:, 0:F], v[:, 2 : F + 2])
            s = spool.tile([128, F], DT16, tag="s")
            nc.vector.tensor_add(s[:], w[:], p12[:])
            junk = jpool.tile([128, F], DT16, tag="junk")
            nc.scalar.activation(
                junk[:], s[:], AF.Square, accum_out=accums[:, col : col + 1]
            )

        # class SSE per row group (fp32, tiny)
        for g in range(G):
            ct, cp = cls_tiles[g]
            cd = cpool.tile([128, 3], dt.float32, tag="cd")
            nc.vector.tensor_sub(cd[:], ct[:], cp[:])
            cj = cpool.tile([128, 3], dt.float32, tag="cj")
            col = G * NT + g
            nc.scalar.activation(
                cj[:], cd[:], AF.Square, accum_out=accums[:, col : col + 1]
            )

        nc.sync.dma_start(out[:], accums[:])

    nc.finalize()
    return nc


_NC = None
last_result = None  # BassKernelResults of the most recent run (for test harness)


def kernel(target_angle, pred_angle, target_class, pred_class):
    global _NC, last_result
    if _NC is None:
        _NC = build_nc()

    in_maps = []
    for c in range(N_CORES):
        r = slice(c * RPC, (c + 1) * RPC)
        in_maps.append(
            {
                "target_angle": np.ascontiguousarray(target_angle[r], dtype=np.float32),
                "pred_angle": np.ascontiguousarray(pred_angle[r], dtype=np.float32),
                "target_class": np.ascontiguousarray(target_class[r], dtype=np.float32),
                "pred_class": np.ascontiguousarray(pred_class[r], dtype=np.float32),
            }
        )

    last_result = run_bass_kernel_spmd(
        _NC,
        in_maps,
        core_ids=list(range(N_CORES)),
        trace=bool(os.environ.get("BASS_TRACE")),
    )

    angle = 0.0
    cls = 0.0
    for r in last_result.results:
        o = np.asarray(r["out"], dtype=np.float64)
        angle += o[:, 0 : G * NT].sum()
        cls += o[:, G * NT : NACC].sum()

    val = 0.8 * (0.05 * 0.05) * angle + 0.2 * cls
    return np.array(val, dtype=np.float32)


# revision 12
# speedup vs baseline: 1.1408x; 1.1339x over previous
"""Trainium2 Bass kernel for nn_D_loss_67551245631962.

Computes: 0.8 * sum(WMA5(target_angle - pred_angle)^2) + 0.2 * sum((target_class - pred_class)^2)
where WMA5 is a 5-tap [0.05, 0.1, 0.7, 0.1, 0.05] correlation with 2-zero padding per side.

Strategy (pure data parallelism over batch dim B=2048 across 8 cores, 256 rows/core):
  Filter factorization: [1,2,14,2,1] = [1,1](*)[1,1](*)[1,0,1] + 12*delta_2.
  Per (group, tile) with 2-col halo each side, d = ta - pa, then
    u = d0 + d1; v = u0 + u1; w = v0 + v2   (DVE TT fp16 2x)  -> [1,2,2,2,1](*)d
    p12 = 12 * d2                           (ACT Copy scale)
    s = w + p12                             (DVE TT)           -> wma / 0.05
    sum(s^2)                                (ACT Square + fp32 accum col)
  Loads: SWDGE cast DMAs (fp32 -> fp16) in 2048-col chunks, ALL descriptor-gens
  emitted before any gpsimd compute so the 16 DMA engines stream continuously.
  Subs run per chunk (DVE mostly; two mid-stream chunks on GpSimd). The last
  tile's chain is split in half-tiles to shorten the post-DMA tail.
  Host applies 0.8*0.05^2 / 0.2 and sums the per-core accumulator columns.
"""

import os
import sys

for _p in ("/opt/trn_rl_repo",):
    if os.path.isdir(_p) and _p not in sys.path:
        sys.path.insert(0, _p)

from contextlib import ExitStack

import numpy as np

import concourse.bass as bass
import concourse.tile as tile
from concourse import bacc, mybir
from concourse.bass_utils import run_bass_kernel_spmd

N_CORES = 8
B, T = 2048, 8192
RPC = B // N_CORES  # rows per core = 256
G = RPC // 128      # 128-partition row groups per core = 2
F = 4096            # conv output cols per tile
NT = T // F         # column tiles per group = 2
H = 2048            # load/sub chunk width (half tile)

DT16 = mybir.dt.float16

# chunk indices (in load order) whose subtraction runs on GpSimd
GPS_SUBS = tuple(
    int(x) for x in os.environ.get("K_GPS_SUBS", "2,3").split(",") if x != ""
)
NACC = None  # set below once SPLIT_TILES known
# tiles (load order) whose chain runs at half-tile granularity (short tail)
SPLIT_TILES = tuple(
    int(x) for x in os.environ.get("K_SPLIT_TILES", "3").split(",") if x != ""
)
NANG = G * NT + len(SPLIT_TILES)  # one accum col per chain invocation
NACC = NANG + G                   # + class cols


def build_nc():
    nc = bacc.Bacc("TRN2")
    dt = mybir.dt
    ta = nc.dram_tensor("target_angle", [RPC, T], dt.float32, kind="ExternalInput")
    pa = nc.dram_tensor("pred_angle", [RPC, T], dt.float32, kind="ExternalInput")
    tcl = nc.dram_tensor("target_class", [RPC, 3], dt.float32, kind="ExternalInput")
    pcl = nc.dram_tensor("pred_class", [RPC, 3], dt.float32, kind="ExternalInput")
    out = nc.dram_tensor("out", [128, NACC], dt.float32, kind="ExternalOutput")

    AF = mybir.ActivationFunctionType
    OP = mybir.AluOpType

    # tiles in load order: (g, t); t-major last so the final tile loads last
    order = [(g, t) for t in range(NT) for g in range(G)]

    with tile.TileContext(nc) as tc, ExitStack() as ctx:
        apool = ctx.enter_context(tc.tile_pool(name="lda", bufs=2 * NT * G))
        bpool = ctx.enter_context(tc.tile_pool(name="ldb", bufs=2 * NT * G))
        dpool = ctx.enter_context(tc.tile_pool(name="d", bufs=3))
        upool = ctx.enter_context(tc.tile_pool(name="u", bufs=2))
        vpool = ctx.enter_context(tc.tile_pool(name="v", bufs=2))
        wpool = ctx.enter_context(tc.tile_pool(name="w", bufs=2))
        ppool = ctx.enter_context(tc.tile_pool(name="p12", bufs=2))
        spool = ctx.enter_context(tc.tile_pool(name="s", bufs=2))
        jpool = ctx.enter_context(tc.tile_pool(name="junk", bufs=2))
        kpool = ctx.enter_context(tc.tile_pool(name="acc", bufs=1))
        cpool = ctx.enter_context(tc.tile_pool(name="cls", bufs=2))

        accums = kpool.tile([128, NACC], dt.float32)

        # chunk geometry: tile (g,t) needs d cols [t*F-2, t*F+F+2) -> two chunks:
        #   chunk 0: d[:, 0:H+4)   <- DRAM cols [t*F-2,     t*F+H+2)
        #   chunk 1: d[:, H+4:F+4) <- DRAM cols [t*F+H+2,   t*F+F+2)
        # (chunk widths H+4 and H-... -> uniform [128, H+4] load tiles, leading cols used)
        def chunk_geom(t, h):
            if h == 0:
                lo, hi = t * F - 2, t * F + H + 2
            else:
                lo, hi = t * F + H + 2, t * F + F + 2
            dst_lo = 0 if h == 0 else H + 4
            pad_l = pad_r = 0
            if lo < 0:
                pad_l, lo = -lo, 0
            if hi > T:
                pad_r, hi = hi - T, T
            return lo, hi, dst_lo, pad_l, pad_r

        # ---- Phase A: all cast-load descriptor gens up-front (gpsimd SWDGE)
        loads = {}  # (g,t,h) -> (A, Bt, width, pad_l, pad_r)
        for (g, t) in order:
            r0, r1 = g * 128, (g + 1) * 128
            for h in (0, 1):
                lo, hi, dst_lo, pad_l, pad_r = chunk_geom(t, h)
                wdt = hi - lo
                A = apool.tile([128, H + 4], DT16, tag="A")
                Bt = bpool.tile([128, H + 4], DT16, tag="B")
                if pad_l:
                    nc.vector.memset(A[:, 0:pad_l], 0.0)
                    nc.vector.memset(Bt[:, 0:pad_l], 0.0)
                if pad_r:
                    nc.vector.memset(A[:, pad_l + wdt : pad_l + wdt + pad_r], 0.0)
                    nc.vector.memset(Bt[:, pad_l + wdt : pad_l + wdt + pad_r], 0.0)
                nc.gpsimd.dma_start(A[:, pad_l : pad_l + wdt], ta[r0:r1, lo:hi])
                nc.gpsimd.dma_start(Bt[:, pad_l : pad_l + wdt], pa[r0:r1, lo:hi])
                loads[(g, t, h)] = (A, Bt, pad_l + wdt + pad_r, dst_lo)

        # class loads early (HWDGE sync queue, fp32)
        cls_tiles = []
        for g in range(G):
            r0, r1 = g * 128, (g + 1) * 128
            ct = cpool.tile([128, 3], dt.float32, tag="ct")
            cp = cpool.tile([128, 3], dt.float32, tag="cp")
            nc.sync.dma_start(ct[:], tcl[r0:r1, :])
            nc.sync.dma_start(cp[:], pcl[r0:r1, :])
            cls_tiles.append((ct, cp))

        # ---- Phase B/C: subs per chunk + chain per tile, in load order
        dtiles = {}

        def emit_sub(ti, g, t, h):
            chunk_idx = 2 * ti + h
            A, Bt, wdt, dst_lo = loads[(g, t, h)]
            d = dtiles[(g, t)]
            dst = d[:, dst_lo : dst_lo + wdt]
            if chunk_idx in GPS_SUBS:
                gps_subs_pend.append((dst, A, Bt, wdt))
            else:
                nc.vector.tensor_sub(dst, A[:, 0:wdt], Bt[:, 0:wdt])

        def emit_chain(g, t, c0, w_out):
            # conv outputs [t*F+c0, t*F+c0+w_out) using d[:, c0 : c0+w_out+4)
            d = dtiles[(g, t)]
            col = next_col[0]
            next_col[0] += 1
            p12 = ppool.tile([128, w_out], DT16, tag="p12")
            nc.scalar.activation(p12[:], d[:, c0 + 2 : c0 + 2 + w_out], AF.Copy, scale=12.0)
            u = upool.tile([128, w_out + 3], DT16, tag="u")
            nc.vector.tensor_add(u[:], d[:, c0 : c0 + w_out + 3], d[:, c0 + 1 : c0 + w_out + 4])
            v = vpool.tile([128, w_out + 2], DT16, tag="v")
            nc.vector.tensor_add(v[:], u[:, 0 : w_out + 2], u[:, 1 : w_out + 3])
            w = wpool.tile([128, w_out], DT16, tag="w")
            nc.vector.tensor_add(w[:], v[:, 0:w_out], v[:, 2 : w_out + 2])
            s = spool.tile([128, w_out], DT16, tag="s")
            nc.vector.tensor_add(s[:], w[:], p12[:])
            junk = jpool.tile([128, w_out], DT16, tag="junk")
            nc.scalar.activation(
                junk[:], s[:], AF.Square, accum_out=accums[:, col : col + 1]
            )

        next_col = [0]
        gps_subs_pend = []
        chains_pend = []
        for ti, (g, t) in enumerate(order):
            d_tile = dpool.tile([128, F + 4], DT16, tag="d")
            dtiles[(g, t)] = d_tile
            emit_sub(ti, g, t, 0)
            emit_sub(ti, g, t, 1)
            chains_pend.append((ti, g, t))

        # gpsimd subs now (after ALL descriptor gens in the gpsimd stream)
        for dst, A, Bt, wdt in gps_subs_pend:
            nc.gpsimd.tensor_tensor(dst, A[:, 0:wdt], Bt[:, 0:wdt], OP.subtract)

        # chains in load order
        for ti, g, t in chains_pend:
            if ti in SPLIT_TILES:
                emit_chain(g, t, 0, H)
                emit_chain(g, t, H, F - H)
            else:
                emit_chain(g, t, 0, F)

        # class SSE per row group (fp32, tiny)
        for g in range(G):
            ct, cp = cls_tiles[g]
            cd = cpool.tile([128, 3], dt.float32, tag="cd")
            nc.vector.tensor_sub(cd[:], ct[:], cp[:])
            cj = cpool.tile([128, 3], dt.float32, tag="cj")
            col = NANG + g
            nc.scalar.activation(
                cj[:], cd[:], AF.Square, accum_out=accums[:, col : col + 1]
            )

        nc.sync.dma_start(out[:], accums[:])

    nc.finalize()
    return nc


_NC = None
last_result = None  # BassKernelResults of the most recent run (for test harness)


def kernel(target_angle, pred_angle, target_class, pred_class):
    global _NC, last_result
    if _NC is None:
        _NC = build_nc()

    in_maps = []
    for c in range(N_CORES):
        r = slice(c * RPC, (c + 1) * RPC)
        in_maps.append(
            {
                "target_angle": np.ascontiguousarray(target_angle[r], dtype=np.float32),
                "pred_angle": np.ascontiguousarray(pred_angle[r], dtype=np.float32),
                "target_class": np.ascontiguousarray(target_class[r], dtype=np.float32),
                "pred_class": np.ascontiguousarray(pred_class[r], dtype=np.float32),
            }
        )

    last_result = run_bass_kernel_spmd(
        _NC,
        in_maps,
        core_ids=list(range(N_CORES)),
        trace=bool(os.environ.get("BASS_TRACE")),
    )

    angle = 0.0
    cls = 0.0
    for r in last_result.results:
        o = np.asarray(r["out"], dtype=np.float64)
        angle += o[:, 0:NANG].sum()
        cls += o[:, NANG:NACC].sum()

    val = 0.8 * (0.05 * 0.05) * angle + 0.2 * cls
    return np.array(val, dtype=np.float32)
